# revision 2
# baseline (speedup 1.0000x reference)
"""Fused self-attention kernel for Trainium2 (Bass/Tile), SPMD over 8 cores.

Math (per batch b):
    q = x @ Wq + bq ; k = x @ Wk + bk ; v = x @ Wv + bv          [T, C]
    scores[t, s] = k[t] . q[s]      (non-causal, unscaled)
    beta = softmax(scores, axis=s)
    attn[t] = sum_s beta[t, s] * v[s]
    out = gamma * attn + x

Sharding: 8 cores = 4 batches x 2 halves of the output rows t. Each core
receives its batch's x rotated so its local 2048 output rows come first
(softmax/attention over s is permutation invariant, so rotating s is safe).
All cores run the identical program on different data.

On-chip layout: scoresT[s, t] = qT.T @ kT is computed with s on partitions
and t on the free axis; the softmax denominator comes for free by appending
a ones column to V (attn_aug = [V | 1].T @ exp(scoresT)).  No max-subtraction
is needed: |scores| < ~60 for any remotely normalized input, and exp is
evaluated in fp32 (overflow threshold 88).  The T x T score matrix never
touches HBM.
"""

import numpy as np
from contextlib import ExitStack

import concourse.bass as bass
import concourse.tile as tile
from concourse import bacc, mybir
from concourse.bass_utils import run_bass_kernel_spmd
from concourse.masks import make_identity

FP32 = mybir.dt.float32
BF16 = mybir.dt.bfloat16
AF = mybir.ActivationFunctionType

B, T, C = 4, 4096, 64
CA = C + 1            # x gets a ones column appended (folds biases into matmuls)
HALVES = 2            # cores per batch
N_CORES = B * HALVES
T_LOC = T // HALVES   # output rows per core
P = 128
NT = T // P           # 32 s-tiles of 128
TB = 512              # t-block width (one PSUM bank of fp32)
N_TB = T_LOC // TB    # 4
SB = 512              # qT/kT column chunk width


def _emit(tc, ctx, x_d, wq_d, wk_d, wv_d, bq_d, bk_d, bv_d, g_d, out_d):
    nc = tc.nc

    const = ctx.enter_context(tc.tile_pool(name="const", bufs=1))
    setup = ctx.enter_context(tc.tile_pool(name="setup", bufs=2))
    expp = ctx.enter_context(tc.tile_pool(name="expp", bufs=3))
    osbp = ctx.enter_context(tc.tile_pool(name="osbp", bufs=2))
    outp = ctx.enter_context(tc.tile_pool(name="outp", bufs=4))
    smallp = ctx.enter_context(tc.tile_pool(name="smallp", bufs=4))
    ps_big = ctx.enter_context(tc.tile_pool(name="ps_big", bufs=3, space="PSUM"))
    ps_o = ctx.enter_context(tc.tile_pool(name="ps_o", bufs=2, space="PSUM"))
    ps_t = ctx.enter_context(tc.tile_pool(name="ps_t", bufs=2, space="PSUM"))

    # ---- constants ------------------------------------------------------
    ident = const.tile([P, P], FP32, tag="ident")
    make_identity(nc, ident)

    g128 = const.tile([P, 1], FP32, tag="g128")
    nc.sync.dma_start(g128, g_d.ap().to_broadcast([P, 1]))

    def w_aug(w_d, b_d, name):
        # [CA, C] bf16: weight with its bias appended as row C (the ones
        # column of x_aug multiplies it back in).
        w = const.tile([CA, C], BF16, tag=name)
        tw = setup.tile([C, C], FP32, tag="tw")
        nc.sync.dma_start(tw, w_d.ap())
        nc.vector.tensor_copy(w[0:C, :], tw)
        tb_ = setup.tile([1, C], FP32, tag="tb")
        nc.sync.dma_start(tb_, b_d.ap()[None, :])
        nc.vector.tensor_copy(w[C:CA, :], tb_)
        return w

    wq = w_aug(wq_d, bq_d, "wq")
    wk = w_aug(wk_d, bk_d, "wk")
    wv = w_aug(wv_d, bv_d, "wv")

    # ---- load x, build xT ----------------------------------------------
    x_v = x_d.ap().rearrange("(n p) c -> p n c", p=P)  # [128, 32, 65]
    x_nat = const.tile([P, NT, CA], FP32, tag="xnat")
    for i in range(4):
        nc.sync.dma_start(x_nat[:, i * 8:(i + 1) * 8, :], x_v[:, i * 8:(i + 1) * 8, :])

    xT = const.tile([CA, T], BF16, tag="xT")  # [65, 4096] bf16 (row 64 = ones)
    for g in range(T // TB):
        psx = ps_big.tile([P, TB], FP32, tag="big")
        for j in range(TB // P):
            idx = g * 4 + j
            nc.tensor.transpose(psx[0:CA, j * P:(j + 1) * P], x_nat[:, idx, :], ident)
        nc.vector.tensor_copy(xT[:, g * TB:(g + 1) * TB], psx[0:CA, :])

    # ---- projections ----------------------------------------------------
    # qT[d, s] over all s; kT[d, t] over local t; v_aug[s, C+1] over all s.
    qt = []
    for i in range(T // SB):
        ps = ps_big.tile([P, SB], FP32, tag="big")
        nc.tensor.matmul(ps[0:C, :], lhsT=wq, rhs=xT[:, i * SB:(i + 1) * SB],
                         start=True, stop=True)
        q_sb = const.tile([C, SB], BF16, tag=f"qt{i}")
        nc.vector.tensor_copy(q_sb, ps[0:C, :])
        qt.append(q_sb)

    kt = []
    for i in range(T_LOC // SB):
        ps = ps_big.tile([P, SB], FP32, tag="big")
        nc.tensor.matmul(ps[0:C, :], lhsT=wk, rhs=xT[:, i * SB:(i + 1) * SB],
                         start=True, stop=True)
        k_sb = const.tile([C, SB], BF16, tag=f"kt{i}")
        nc.vector.tensor_copy(k_sb, ps[0:C, :])
        kt.append(k_sb)

    va = []
    for g in range(NT // 8):
        ps = ps_big.tile([P, 8 * C], FP32, tag="big")
        for j in range(8):
            idx = g * 8 + j
            nc.tensor.matmul(ps[:, j * C:(j + 1) * C],
                             lhsT=xT[:, idx * P:(idx + 1) * P], rhs=wv,
                             start=True, stop=True)
        v_sb = const.tile([P, 8, CA], BF16, tag=f"va{g}")
        nc.vector.tensor_copy(v_sb[:, :, 0:C], ps.rearrange("p (n c) -> p n c", c=C))
        nc.vector.memset(v_sb[:, :, C:CA], 1.0)
        va.append(v_sb)

    # ---- flash attention main loop --------------------------------------
    # Software-pipelined: scores matmul for iteration i+1 is emitted before
    # the attn matmul for iteration i so PE never waits on ACT's exp.
    out_v = out_d.ap().rearrange("(n p) c -> p n c", p=P)  # [128, 16, 64]

    for tb in range(N_TB):
        po = ps_o.tile([CA, TB], FP32, tag="o")  # [V|1].T @ exp, accumulated
        pss = [None] * NT
        ex = [None] * NT

        def scores(st):
            pss[st] = ps_big.tile([P, TB], FP32, tag="big", name="pss")
            nc.tensor.matmul(pss[st], lhsT=qt[st // 4][:, (st % 4) * P:(st % 4 + 1) * P],
                             rhs=kt[tb], start=True, stop=True)
            ex[st] = expp.tile([P, TB], BF16, tag="ex", name="ex")
            nc.scalar.activation(ex[st], pss[st], AF.Exp)

        def attn(st):
            nc.tensor.matmul(po, lhsT=va[st // 8][:, st % 8, :], rhs=ex[st],
                             start=(st == 0), stop=(st == NT - 1))

        scores(0)
        for st in range(1, NT):
            scores(st)
            attn(st - 1)
        attn(NT - 1)

        # finalize: transpose [CA, 128] chunks back to [128, CA], normalize,
        # apply gamma, add residual, store.
        osb = osbp.tile([CA, TB], FP32, tag="osb")
        nc.vector.tensor_copy(osb, po)
        for j in range(TB // P):
            pt = ps_t.tile([P, CA], FP32, tag="t")
            nc.tensor.transpose(pt, osb[:, j * P:(j + 1) * P], ident[0:CA, 0:CA])
            rec = smallp.tile([P, 1], FP32, tag="rec")
            nc.vector.reciprocal(rec, pt[:, C:CA])
            grec = smallp.tile([P, 1], FP32, tag="grec")
            nc.vector.tensor_mul(grec, rec, g128)
            ot = outp.tile([P, C], FP32, tag="ot")
            nc.vector.tensor_scalar_mul(ot, pt[:, 0:C], grec)
            idx = tb * (TB // P) + j
            nc.vector.tensor_add(ot, ot, x_nat[:, idx, 0:C])
            nc.sync.dma_start(out_v[:, idx, :], ot)


def build():
    nc = bacc.Bacc("TRN2", target_bir_lowering=False, debug=False,
                   num_devices=N_CORES)
    x_d = nc.dram_tensor("x", [T, CA], FP32, kind="ExternalInput")
    wq_d = nc.dram_tensor("wq", [C, C], FP32, kind="ExternalInput")
    wk_d = nc.dram_tensor("wk", [C, C], FP32, kind="ExternalInput")
    wv_d = nc.dram_tensor("wv", [C, C], FP32, kind="ExternalInput")
    bq_d = nc.dram_tensor("bq", [C], FP32, kind="ExternalInput")
    bk_d = nc.dram_tensor("bk", [C], FP32, kind="ExternalInput")
    bv_d = nc.dram_tensor("bv", [C], FP32, kind="ExternalInput")
    g_d = nc.dram_tensor("gamma", [1], FP32, kind="ExternalInput")
    out_d = nc.dram_tensor("out", [T_LOC, C], FP32, kind="ExternalOutput")

    with tile.TileContext(nc) as tc, ExitStack() as ctx:
        _emit(tc, ctx, x_d, wq_d, wk_d, wv_d, bq_d, bk_d, bv_d, g_d, out_d)
    nc.compile()
    return nc


def make_in_maps(inputs, Wq, bq, Wk, bk, Wv, bv, gamma):
    """Shard the full inputs into per-core input maps."""
    x = np.asarray(inputs, dtype=np.float32).reshape(B, T, C)
    ones = np.ones((T, 1), dtype=np.float32)
    in_maps = []
    for core in range(N_CORES):
        b, h = divmod(core, HALVES)
        xb = x[b]
        if h:
            xb = np.concatenate([xb[h * T_LOC:], xb[:h * T_LOC]], axis=0)
        x_aug = np.ascontiguousarray(np.concatenate([xb, ones], axis=1))
        in_maps.append({
            "x": x_aug,
            "wq": np.asarray(Wq, np.float32), "bq": np.asarray(bq, np.float32),
            "wk": np.asarray(Wk, np.float32), "bk": np.asarray(bk, np.float32),
            "wv": np.asarray(Wv, np.float32), "bv": np.asarray(bv, np.float32),
            "gamma": np.asarray(gamma, np.float32),
        })
    return in_maps


def assemble(results):
    """Gather per-core [T_LOC, C] outputs into the full [B, 1, T, C]."""
    out = np.empty((B, 1, T, C), dtype=np.float32)
    for core in range(N_CORES):
        b, h = divmod(core, HALVES)
        out[b, 0, h * T_LOC:(h + 1) * T_LOC, :] = results[core]["out"]
    return out


_NC_CACHE = []


def kernel(inputs, Wq, bq, Wk, bk, Wv, bv, gamma):
    if not _NC_CACHE:
        _NC_CACHE.append(build())
    nc = _NC_CACHE[0]
    in_maps = make_in_maps(inputs, Wq, bq, Wk, bk, Wv, bv, gamma)
    res = run_bass_kernel_spmd(nc, in_maps, list(range(N_CORES)))
    return assemble(res.results)


# revision 12
# speedup vs baseline: 1.0329x; 1.0329x over previous
"""Fused self-attention kernel for Trainium2 (Bass/Tile), SPMD over 8 cores.

Math (per batch b):
    q = x @ Wq + bq ; k = x @ Wk + bk ; v = x @ Wv + bv          [T, C]
    scores[t, s] = k[t] . q[s]      (non-causal, unscaled)
    beta = softmax(scores, axis=s)
    attn[t] = sum_s beta[t, s] * v[s]
    out = gamma * attn + x

Sharding: 8 cores = 4 batches x 2 halves of the output rows t. Each core
receives its batch's x rotated so its local 2048 output rows come first
(softmax/attention over s is permutation invariant, so rotating s is safe).
All cores run the identical program on different data.

On-chip layout: scoresT[s, t] = qT.T @ kT is computed with s on partitions
and t on the free axis; the softmax denominator comes for free by appending
a ones column to V (attn_aug = [V | 1].T @ exp(scoresT)).  No max-subtraction
is needed: |scores| < ~60 for any remotely normalized input, and exp is
evaluated in fp32 (overflow threshold 88).  The T x T score matrix never
touches HBM.
"""

import numpy as np
from contextlib import ExitStack

import concourse.bass as bass
import concourse.tile as tile
from concourse import bacc, mybir
from concourse.bass_utils import run_bass_kernel_spmd
from concourse.masks import make_identity

FP32 = mybir.dt.float32
BF16 = mybir.dt.bfloat16
AF = mybir.ActivationFunctionType

B, T, C = 4, 4096, 64
CA = C + 1            # x gets a ones column appended (folds biases into matmuls)
HALVES = 2            # cores per batch
N_CORES = B * HALVES
T_LOC = T // HALVES   # output rows per core
P = 128
NT = T // P           # 32 s-tiles of 128
TB = 1024             # t-block width (two PSUM banks; bf16 moving max)
N_TB = T_LOC // TB    # 2
SB = 512              # qT column chunk width
NT_MAIN = NT          # s-tiles processed in the main loop (debug knob)


def _emit(tc, ctx, x_d, wq_d, wk_d, wv_d, bq_d, bk_d, bv_d, g_d, out_d):
    nc = tc.nc

    const = ctx.enter_context(tc.tile_pool(name="const", bufs=1))
    setup = ctx.enter_context(tc.tile_pool(name="setup", bufs=2))
    expp = ctx.enter_context(tc.tile_pool(name="expp", bufs=4))
    osbp = ctx.enter_context(tc.tile_pool(name="osbp", bufs=2))
    outp = ctx.enter_context(tc.tile_pool(name="outp", bufs=4))
    smallp = ctx.enter_context(tc.tile_pool(name="smallp", bufs=4))
    # PSUM budget (8 banks): scores [128,1024] x2 bufs = 4, the two
    # persistent attn accumulators [65,1024] = 4.  The finalize-phase
    # transpose tiles share the scores tag (scores allocation has stopped
    # by then).
    ps_big = ctx.enter_context(tc.tile_pool(name="ps_big", bufs=2, space="PSUM"))
    ps_o = ctx.enter_context(tc.tile_pool(name="ps_o", bufs=1, space="PSUM"))

    # ---- constants ------------------------------------------------------
    ident = const.tile([P, P], FP32, tag="ident")
    make_identity(nc, ident)

    g128 = const.tile([P, 1], FP32, tag="g128")
    nc.sync.dma_start(g128, g_d.ap().to_broadcast([P, 1]))

    def w_aug(w_d, b_d, name):
        # [CA, C] bf16: weight with its bias appended as row C (the ones
        # column of x_aug multiplies it back in).
        w = const.tile([CA, C], BF16, tag=name)
        tw = setup.tile([C, C], FP32, tag="tw")
        nc.sync.dma_start(tw, w_d.ap())
        nc.vector.tensor_copy(w[0:C, :], tw)
        tb_ = setup.tile([1, C], FP32, tag="tb")
        nc.sync.dma_start(tb_, b_d.ap()[None, :])
        nc.vector.tensor_copy(w[C:CA, :], tb_)
        return w

    wq = w_aug(wq_d, bq_d, "wq")
    wk = w_aug(wk_d, bk_d, "wk")
    wv = w_aug(wv_d, bv_d, "wv")

    # ---- load x, build xT ----------------------------------------------
    x_v = x_d.ap().rearrange("(n p) c -> p n c", p=P)  # [128, 32, 65]
    x_nat = const.tile([P, NT, CA], FP32, tag="xnat")
    for i in range(4):
        nc.sync.dma_start(x_nat[:, i * 8:(i + 1) * 8, :], x_v[:, i * 8:(i + 1) * 8, :])

    xT = const.tile([CA, T], BF16, tag="xT")  # [65, 4096] bf16 (row 64 = ones)
    for g in range(T // TB):
        psx = ps_big.tile([P, TB], FP32, tag="big")
        for j in range(TB // P):
            idx = g * (TB // P) + j
            nc.tensor.transpose(psx[0:CA, j * P:(j + 1) * P], x_nat[:, idx, :], ident)
        nc.vector.tensor_copy(xT[:, g * TB:(g + 1) * TB], psx[0:CA, :])

    # ---- projections ----------------------------------------------------
    # qT[d, s] over all s; kT[d, t] over local t; v_aug[s, C+1] over all s.
    qt = []
    for i in range(T // SB):
        ps = ps_big.tile([P, SB], FP32, tag="big")
        nc.tensor.matmul(ps[0:C, :], lhsT=wq, rhs=xT[:, i * SB:(i + 1) * SB],
                         start=True, stop=True)
        q_sb = const.tile([C, SB], BF16, tag=f"qt{i}")
        nc.vector.tensor_copy(q_sb, ps[0:C, :])
        qt.append(q_sb)

    kt = []
    for i in range(T_LOC // TB):
        k_sb = const.tile([C, TB], BF16, tag=f"kt{i}")
        for j in range(TB // SB):
            ps = ps_big.tile([P, SB], FP32, tag="big")
            nc.tensor.matmul(ps[0:C, :], lhsT=wk,
                             rhs=xT[:, i * TB + j * SB:i * TB + (j + 1) * SB],
                             start=True, stop=True)
            nc.vector.tensor_copy(k_sb[:, j * SB:(j + 1) * SB], ps[0:C, :])
        kt.append(k_sb)

    va = []
    for g in range(NT // 8):
        ps = ps_big.tile([P, 8 * C], FP32, tag="big")
        for j in range(8):
            idx = g * 8 + j
            nc.tensor.matmul(ps[:, j * C:(j + 1) * C],
                             lhsT=xT[:, idx * P:(idx + 1) * P], rhs=wv,
                             start=True, stop=True)
        v_sb = const.tile([P, 8, CA], BF16, tag=f"va{g}")
        nc.vector.tensor_copy(v_sb[:, :, 0:C], ps.rearrange("p (n c) -> p n c", c=C))
        nc.vector.memset(v_sb[:, :, C:CA], 1.0)
        va.append(v_sb)

    # ---- flash attention main loop --------------------------------------
    # s-tile outer loop: per s-tile load qt/va stationary weights once and
    # stream both 1024-wide t-blocks; both attn accumulators are persistent
    # in PSUM.  Software-pipelined: scores for s-tile st+1 are emitted before
    # the attn matmuls of s-tile st so PE never waits on ACT's exp.
    out_v = out_d.ap().rearrange("(n p) c -> p n c", p=P)  # [128, 16, 64]

    po = [ps_o.tile([CA, TB], FP32, tag=f"o{tb}", name="po") for tb in range(N_TB)]
    ex = [[None] * N_TB for _ in range(NT)]

    def scores(st):
        for tb in range(N_TB):
            pss = ps_big.tile([P, TB], FP32, tag="big", name="pss")
            for h in range(TB // SB):
                nc.tensor.matmul(
                    pss[:, h * SB:(h + 1) * SB],
                    lhsT=qt[st // 4][:, (st % 4) * P:(st % 4 + 1) * P],
                    rhs=kt[tb][:, h * SB:(h + 1) * SB], start=True, stop=True)
            e = expp.tile([P, TB], BF16, tag="ex", name="ex")
            nc.scalar.activation(e, pss, AF.Exp)
            ex[st][tb] = e

    def attn(st):
        for tb in range(N_TB):
            for h in range(TB // SB):  # matmul output must stay in one PSUM bank
                nc.tensor.matmul(po[tb][:, h * SB:(h + 1) * SB],
                                 lhsT=va[st // 8][:, st % 8, :],
                                 rhs=ex[st][tb][:, h * SB:(h + 1) * SB],
                                 start=(st == 0), stop=(st == NT_MAIN - 1))

    scores(0)
    for st in range(1, NT_MAIN):
        scores(st)
        attn(st - 1)
    attn(NT_MAIN - 1)

    # ---- finalize: transpose [CA, 128] chunks back to [128, CA],
    # normalize, apply gamma, add residual, store.
    for tb in range(N_TB):
        osb = osbp.tile([CA, TB], FP32, tag="osb")
        nc.vector.tensor_copy(osb, po[tb])
        for j in range(TB // P):
            pt = ps_big.tile([P, CA], FP32, tag="big", name="pt")
            nc.tensor.transpose(pt, osb[:, j * P:(j + 1) * P], ident[0:CA, 0:CA])
            rec = smallp.tile([P, 1], FP32, tag="rec")
            nc.vector.reciprocal(rec, pt[:, C:CA])
            grec = smallp.tile([P, 1], FP32, tag="grec")
            nc.vector.tensor_mul(grec, rec, g128)
            ot = outp.tile([P, C], FP32, tag="ot")
            nc.vector.tensor_scalar_mul(ot, pt[:, 0:C], grec)
            idx = tb * (TB // P) + j
            nc.vector.tensor_add(ot, ot, x_nat[:, idx, 0:C])
            nc.sync.dma_start(out_v[:, idx, :], ot)


def build():
    nc = bacc.Bacc("TRN2", target_bir_lowering=False, debug=False,
                   num_devices=N_CORES)
    x_d = nc.dram_tensor("x", [T, CA], FP32, kind="ExternalInput")
    wq_d = nc.dram_tensor("wq", [C, C], FP32, kind="ExternalInput")
    wk_d = nc.dram_tensor("wk", [C, C], FP32, kind="ExternalInput")
    wv_d = nc.dram_tensor("wv", [C, C], FP32, kind="ExternalInput")
    bq_d = nc.dram_tensor("bq", [C], FP32, kind="ExternalInput")
    bk_d = nc.dram_tensor("bk", [C], FP32, kind="ExternalInput")
    bv_d = nc.dram_tensor("bv", [C], FP32, kind="ExternalInput")
    g_d = nc.dram_tensor("gamma", [1], FP32, kind="ExternalInput")
    out_d = nc.dram_tensor("out", [T_LOC, C], FP32, kind="ExternalOutput")

    with tile.TileContext(nc) as tc, ExitStack() as ctx:
        _emit(tc, ctx, x_d, wq_d, wk_d, wv_d, bq_d, bk_d, bv_d, g_d, out_d)
    nc.compile()
    return nc


def make_in_maps(inputs, Wq, bq, Wk, bk, Wv, bv, gamma):
    """Shard the full inputs into per-core input maps."""
    x = np.asarray(inputs, dtype=np.float32).reshape(B, T, C)
    ones = np.ones((T, 1), dtype=np.float32)
    in_maps = []
    for core in range(N_CORES):
        b, h = divmod(core, HALVES)
        xb = x[b]
        if h:
            xb = np.concatenate([xb[h * T_LOC:], xb[:h * T_LOC]], axis=0)
        x_aug = np.ascontiguousarray(np.concatenate([xb, ones], axis=1))
        in_maps.append({
            "x": x_aug,
            "wq": np.asarray(Wq, np.float32), "bq": np.asarray(bq, np.float32),
            "wk": np.asarray(Wk, np.float32), "bk": np.asarray(bk, np.float32),
            "wv": np.asarray(Wv, np.float32), "bv": np.asarray(bv, np.float32),
            "gamma": np.asarray(gamma, np.float32),
        })
    return in_maps


def assemble(results):
    """Gather per-core [T_LOC, C] outputs into the full [B, 1, T, C]."""
    out = np.empty((B, 1, T, C), dtype=np.float32)
    for core in range(N_CORES):
        b, h = divmod(core, HALVES)
        out[b, 0, h * T_LOC:(h + 1) * T_LOC, :] = results[core]["out"]
    return out


_NC_CACHE = []


def kernel(inputs, Wq, bq, Wk, bk, Wv, bv, gamma):
    if not _NC_CACHE:
        _NC_CACHE.append(build())
    nc = _NC_CACHE[0]
    in_maps = make_in_maps(inputs, Wq, bq, Wk, bk, Wv, bv, gamma)
    res = run_bass_kernel_spmd(nc, in_maps, list(range(N_CORES)))
    return assemble(res.results)


# revision 15
# speedup vs baseline: 1.2891x; 1.2479x over previous
"""Fused self-attention kernel for Trainium2 (Bass/Tile), SPMD over 8 cores.

Math (per batch b):
    q = x @ Wq + bq ; k = x @ Wk + bk ; v = x @ Wv + bv          [T, C]
    scores[t, s] = k[t] . q[s]      (non-causal, unscaled)
    beta = softmax(scores, axis=s)
    attn[t] = sum_s beta[t, s] * v[s]
    out = gamma * attn + x

Sharding: 8 cores = 4 batches x 2 halves of the output rows t. Each core
receives its batch's x rotated so its local 2048 output rows come first
(softmax/attention over s is permutation invariant, so rotating s is safe).
All cores run the identical program on different data.

On-chip layout: scoresT[s, t] = qT.T @ kT is computed with s on partitions
and t on the free axis; the softmax denominator comes for free by appending
a ones column to V (attn_aug = [V | 1].T @ exp(scoresT)).  No max-subtraction
is needed: |scores| < ~60 for any remotely normalized input, and exp is
evaluated in fp32 (overflow threshold 88).  The T x T score matrix never
touches HBM.
"""

import numpy as np
from contextlib import ExitStack

import concourse.bass as bass
import concourse.tile as tile
from concourse import bacc, mybir
from concourse.bass_utils import run_bass_kernel_spmd
from concourse.masks import make_identity

FP32 = mybir.dt.float32
BF16 = mybir.dt.bfloat16
AF = mybir.ActivationFunctionType

B, T, C = 4, 4096, 64
CA = C + 1            # x gets a ones column appended (folds biases into matmuls)
HALVES = 2            # cores per batch
N_CORES = B * HALVES
T_LOC = T // HALVES   # output rows per core
P = 128
NT = T // P           # 32 s-tiles of 128
TB = 1024             # t-block width (two PSUM banks; bf16 moving max)
N_TB = T_LOC // TB    # 2
SB = 512              # qT column chunk width
NT_MAIN = NT          # s-tiles processed in the main loop (debug knob)


def _emit(tc, ctx, x_d, wq_d, wk_d, wv_d, bq_d, bk_d, bv_d, g_d, out_d):
    nc = tc.nc

    const = ctx.enter_context(tc.tile_pool(name="const", bufs=1))
    setup = ctx.enter_context(tc.tile_pool(name="setup", bufs=2))
    expp = ctx.enter_context(tc.tile_pool(name="expp", bufs=4))
    osbp = ctx.enter_context(tc.tile_pool(name="osbp", bufs=2))
    outp = ctx.enter_context(tc.tile_pool(name="outp", bufs=4))
    smallp = ctx.enter_context(tc.tile_pool(name="smallp", bufs=4))
    # PSUM budget (8 banks): scores [128,1024] x2 bufs = 4, the two
    # persistent attn accumulators [65,1024] = 4.  The finalize-phase
    # transpose tiles share the scores tag (scores allocation has stopped
    # by then).
    ps_big = ctx.enter_context(tc.tile_pool(name="ps_big", bufs=2, space="PSUM"))
    ps_o = ctx.enter_context(tc.tile_pool(name="ps_o", bufs=1, space="PSUM"))

    # ---- constants ------------------------------------------------------
    ident = const.tile([P, P], FP32, tag="ident")
    make_identity(nc, ident)

    g128 = const.tile([P, 1], FP32, tag="g128")
    nc.sync.dma_start(g128, g_d.ap().to_broadcast([P, 1]))

    def w_aug(w_d, b_d, name):
        # [128, 128] bf16, zero padded: rows 0:C = W, row C = bias (the ones
        # column of x_aug multiplies it back in), rest zero.  Full-K/M shapes
        # keep the PE HAM activity monitor seeing full-array matmuls (K<128
        # matmuls never un-throttle the 1.2->2.4 GHz clock gate).
        w = const.tile([P, P], BF16, tag=name)
        nc.vector.memset(w, 0.0)
        tw = setup.tile([C, C], FP32, tag="tw")
        nc.sync.dma_start(tw, w_d.ap())
        nc.vector.tensor_copy(w[0:C, 0:C], tw)
        tb_ = setup.tile([1, C], FP32, tag="tb")
        nc.sync.dma_start(tb_, b_d.ap()[None, :])
        nc.vector.tensor_copy(w[C:CA, 0:C], tb_)
        return w

    wq = w_aug(wq_d, bq_d, "wq")
    wk = w_aug(wk_d, bk_d, "wk")
    wv = w_aug(wv_d, bv_d, "wv")

    # ---- load x, build xT ----------------------------------------------
    x_v = x_d.ap().rearrange("(n p) c -> p n c", p=P)  # [128, 32, 65]
    x_nat = const.tile([P, NT, CA], FP32, tag="xnat")
    for i in range(4):
        nc.sync.dma_start(x_nat[:, i * 8:(i + 1) * 8, :], x_v[:, i * 8:(i + 1) * 8, :])

    xT = const.tile([P, T], BF16, tag="xT")  # rows: 0:C x.T, C ones, rest 0
    # zero the pad rows (64:128); the ones row (64) is rewritten by the
    # transpose copies below.  gpsimd wants 32-aligned start partitions.
    nc.gpsimd.memset(xT[C:P, :], 0.0)
    for g in range(T // TB):
        psx = ps_big.tile([P, TB], FP32, tag="big")
        for j in range(TB // P):
            idx = g * (TB // P) + j
            nc.tensor.transpose(psx[0:CA, j * P:(j + 1) * P], x_nat[:, idx, :], ident)
        nc.vector.tensor_copy(xT[0:CA, g * TB:(g + 1) * TB], psx[0:CA, :])

    # ---- projections ----------------------------------------------------
    # qT[d, s] over all s; kT[d, t] over local t; v_aug[s, C+1] over all s.
    qt = []
    for i in range(T // SB):
        ps = ps_big.tile([P, SB], FP32, tag="big")
        nc.tensor.matmul(ps, lhsT=wq, rhs=xT[:, i * SB:(i + 1) * SB],
                         start=True, stop=True)
        q_sb = const.tile([P, SB], BF16, tag=f"qt{i}")
        nc.vector.tensor_copy(q_sb, ps)
        qt.append(q_sb)

    kt = []
    for i in range(T_LOC // TB):
        k_sb = const.tile([P, TB], BF16, tag=f"kt{i}")
        for j in range(TB // SB):
            ps = ps_big.tile([P, SB], FP32, tag="big")
            nc.tensor.matmul(ps, lhsT=wk,
                             rhs=xT[:, i * TB + j * SB:i * TB + (j + 1) * SB],
                             start=True, stop=True)
            nc.vector.tensor_copy(k_sb[:, j * SB:(j + 1) * SB], ps)
        kt.append(k_sb)

    va = []
    for g in range(NT // 8):
        ps = ps_big.tile([P, 8 * C], FP32, tag="big")
        for j in range(8):
            idx = g * 8 + j
            nc.tensor.matmul(ps[:, j * C:(j + 1) * C],
                             lhsT=xT[:, idx * P:(idx + 1) * P], rhs=wv[:, 0:C],
                             start=True, stop=True)
        v_sb = const.tile([P, 8, P], BF16, tag=f"va{g}")
        nc.vector.tensor_copy(v_sb[:, :, 0:C], ps.rearrange("p (n c) -> p n c", c=C))
        nc.vector.memset(v_sb[:, :, C:CA], 1.0)
        nc.vector.memset(v_sb[:, :, CA:P], 0.0)
        va.append(v_sb)

    # ---- flash attention main loop --------------------------------------
    # s-tile outer loop: per s-tile load qt/va stationary weights once and
    # stream both 1024-wide t-blocks; both attn accumulators are persistent
    # in PSUM.  Software-pipelined: scores for s-tile st+1 are emitted before
    # the attn matmuls of s-tile st so PE never waits on ACT's exp.
    out_v = out_d.ap().rearrange("(n p) c -> p n c", p=P)  # [128, 16, 64]

    po = [ps_o.tile([P, TB], FP32, tag=f"o{tb}", name="po") for tb in range(N_TB)]
    ex = [[None] * N_TB for _ in range(NT)]

    def scores(st):
        for tb in range(N_TB):
            pss = ps_big.tile([P, TB], FP32, tag="big", name="pss")
            for h in range(TB // SB):
                nc.tensor.matmul(
                    pss[:, h * SB:(h + 1) * SB],
                    lhsT=qt[st // 4][:, (st % 4) * P:(st % 4 + 1) * P],
                    rhs=kt[tb][:, h * SB:(h + 1) * SB], start=True, stop=True)
            e = expp.tile([P, TB], BF16, tag="ex", name="ex")
            nc.scalar.activation(e, pss, AF.Exp)
            ex[st][tb] = e

    def attn(st):
        for tb in range(N_TB):
            for h in range(TB // SB):  # matmul output must stay in one PSUM bank
                nc.tensor.matmul(po[tb][:, h * SB:(h + 1) * SB],
                                 lhsT=va[st // 8][:, st % 8, :],
                                 rhs=ex[st][tb][:, h * SB:(h + 1) * SB],
                                 start=(st == 0), stop=(st == NT_MAIN - 1))

    scores(0)
    for st in range(1, NT_MAIN):
        scores(st)
        attn(st - 1)
    attn(NT_MAIN - 1)

    # ---- finalize: transpose [CA, 128] chunks back to [128, CA],
    # normalize, apply gamma, add residual, store.
    for tb in range(N_TB):
        osb = osbp.tile([P, TB], FP32, tag="osb")
        nc.vector.tensor_copy(osb, po[tb])
        for j in range(TB // P):
            pt = ps_big.tile([P, P], FP32, tag="big", name="pt")
            nc.tensor.transpose(pt, osb[:, j * P:(j + 1) * P], ident)
            rec = smallp.tile([P, 1], FP32, tag="rec")
            nc.vector.reciprocal(rec, pt[:, C:CA])
            grec = smallp.tile([P, 1], FP32, tag="grec")
            nc.vector.tensor_mul(grec, rec, g128)
            ot = outp.tile([P, C], FP32, tag="ot")
            nc.vector.tensor_scalar_mul(ot, pt[:, 0:C], grec)
            idx = tb * (TB // P) + j
            nc.vector.tensor_add(ot, ot, x_nat[:, idx, 0:C])
            nc.sync.dma_start(out_v[:, idx, :], ot)


def build():
    nc = bacc.Bacc("TRN2", target_bir_lowering=False, debug=False,
                   num_devices=N_CORES)
    x_d = nc.dram_tensor("x", [T, CA], FP32, kind="ExternalInput")
    wq_d = nc.dram_tensor("wq", [C, C], FP32, kind="ExternalInput")
    wk_d = nc.dram_tensor("wk", [C, C], FP32, kind="ExternalInput")
    wv_d = nc.dram_tensor("wv", [C, C], FP32, kind="ExternalInput")
    bq_d = nc.dram_tensor("bq", [C], FP32, kind="ExternalInput")
    bk_d = nc.dram_tensor("bk", [C], FP32, kind="ExternalInput")
    bv_d = nc.dram_tensor("bv", [C], FP32, kind="ExternalInput")
    g_d = nc.dram_tensor("gamma", [1], FP32, kind="ExternalInput")
    out_d = nc.dram_tensor("out", [T_LOC, C], FP32, kind="ExternalOutput")

    with tile.TileContext(nc) as tc, ExitStack() as ctx:
        _emit(tc, ctx, x_d, wq_d, wk_d, wv_d, bq_d, bk_d, bv_d, g_d, out_d)
    nc.compile()
    return nc


def make_in_maps(inputs, Wq, bq, Wk, bk, Wv, bv, gamma):
    """Shard the full inputs into per-core input maps."""
    x = np.asarray(inputs, dtype=np.float32).reshape(B, T, C)
    ones = np.ones((T, 1), dtype=np.float32)
    in_maps = []
    for core in range(N_CORES):
        b, h = divmod(core, HALVES)
        xb = x[b]
        if h:
            xb = np.concatenate([xb[h * T_LOC:], xb[:h * T_LOC]], axis=0)
        x_aug = np.ascontiguousarray(np.concatenate([xb, ones], axis=1))
        in_maps.append({
            "x": x_aug,
            "wq": np.asarray(Wq, np.float32), "bq": np.asarray(bq, np.float32),
            "wk": np.asarray(Wk, np.float32), "bk": np.asarray(bk, np.float32),
            "wv": np.asarray(Wv, np.float32), "bv": np.asarray(bv, np.float32),
            "gamma": np.asarray(gamma, np.float32),
        })
    return in_maps


def assemble(results):
    """Gather per-core [T_LOC, C] outputs into the full [B, 1, T, C]."""
    out = np.empty((B, 1, T, C), dtype=np.float32)
    for core in range(N_CORES):
        b, h = divmod(core, HALVES)
        out[b, 0, h * T_LOC:(h + 1) * T_LOC, :] = results[core]["out"]
    return out


_NC_CACHE = []


def kernel(inputs, Wq, bq, Wk, bk, Wv, bv, gamma):
    if not _NC_CACHE:
        _NC_CACHE.append(build())
    nc = _NC_CACHE[0]
    in_maps = make_in_maps(inputs, Wq, bq, Wk, bk, Wv, bv, gamma)
    res = run_bass_kernel_spmd(nc, in_maps, list(range(N_CORES)))
    return assemble(res.results)


# revision 17
# speedup vs baseline: 1.5442x; 1.1979x over previous
"""Fused self-attention kernel for Trainium2 (Bass/Tile), SPMD over 8 cores.

Math (per batch b):
    q = x @ Wq + bq ; k = x @ Wk + bk ; v = x @ Wv + bv          [T, C]
    scores[t, s] = k[t] . q[s]      (non-causal, unscaled)
    beta = softmax(scores, axis=s)
    attn[t] = sum_s beta[t, s] * v[s]
    out = gamma * attn + x

Sharding: 8 cores = 4 batches x 2 halves of the output rows t. Each core
receives its batch's x rotated so its local 2048 output rows come first
(softmax/attention over s is permutation invariant, so rotating s is safe).
All cores run the identical program on different data.

On-chip layout: scoresT[s, t] = qT.T @ kT is computed with s on partitions
and t on the free axis; the softmax denominator comes for free by appending
a ones column to V (attn_aug = [V | 1].T @ exp(scoresT)).  No max-subtraction
is needed: |scores| < ~60 for any remotely normalized input, and exp is
evaluated in fp32 (overflow threshold 88).  The T x T score matrix never
touches HBM.
"""

import numpy as np
from contextlib import ExitStack

import concourse.bass as bass
import concourse.tile as tile
from concourse import bacc, mybir
from concourse.bass_utils import run_bass_kernel_spmd
from concourse.masks import make_identity

FP32 = mybir.dt.float32
BF16 = mybir.dt.bfloat16
AF = mybir.ActivationFunctionType

B, T, C = 4, 4096, 64
CA = C + 1            # x gets a ones column appended (folds biases into matmuls)
HALVES = 2            # cores per batch
N_CORES = B * HALVES
T_LOC = T // HALVES   # output rows per core
P = 128
NT = T // P           # 32 s-tiles of 128
TB = 1024             # t-block width (two PSUM banks; bf16 moving max)
N_TB = T_LOC // TB    # 2
SB = 512              # qT column chunk width
NT_MAIN = NT          # s-tiles processed in the main loop (debug knob)


def _emit(tc, ctx, x_d, wq_d, wk_d, wv_d, bq_d, bk_d, bv_d, g_d, out_d):
    nc = tc.nc

    const = ctx.enter_context(tc.tile_pool(name="const", bufs=1))
    setup = ctx.enter_context(tc.tile_pool(name="setup", bufs=2))
    expp = ctx.enter_context(tc.tile_pool(name="expp", bufs=6))
    osbp = ctx.enter_context(tc.tile_pool(name="osbp", bufs=2))
    outp = ctx.enter_context(tc.tile_pool(name="outp", bufs=6))
    smallp = ctx.enter_context(tc.tile_pool(name="smallp", bufs=8))
    # PSUM budget (8 banks): scores [128,1024] x2 bufs = 4, the two
    # persistent attn accumulators [65,1024] = 4.  The finalize-phase
    # transpose tiles share the scores tag (scores allocation has stopped
    # by then).
    ps_big = ctx.enter_context(tc.tile_pool(name="ps_big", bufs=2, space="PSUM"))
    ps_o = ctx.enter_context(tc.tile_pool(name="ps_o", bufs=1, space="PSUM"))

    # ---- constants ------------------------------------------------------
    ident = const.tile([P, P], FP32, tag="ident")
    make_identity(nc, ident)

    g128 = const.tile([P, 1], FP32, tag="g128")
    nc.sync.dma_start(g128, g_d.ap().to_broadcast([P, 1]))

    def w_aug(w_d, b_d, name):
        # [128, 128] bf16, zero padded: rows 0:C = W, row C = bias (the ones
        # column of x_aug multiplies it back in), rest zero.  Full-K/M shapes
        # keep the PE HAM activity monitor seeing full-array matmuls (K<128
        # matmuls never un-throttle the 1.2->2.4 GHz clock gate).
        w = const.tile([P, P], BF16, tag=name)
        nc.vector.memset(w, 0.0)
        tw = setup.tile([C, C], FP32, tag="tw")
        nc.sync.dma_start(tw, w_d.ap())
        nc.vector.tensor_copy(w[0:C, 0:C], tw)
        tb_ = setup.tile([1, C], FP32, tag="tb")
        nc.sync.dma_start(tb_, b_d.ap()[None, :])
        nc.vector.tensor_copy(w[C:CA, 0:C], tb_)
        return w

    wq = w_aug(wq_d, bq_d, "wq")
    wk = w_aug(wk_d, bk_d, "wk")
    wv = w_aug(wv_d, bv_d, "wv")

    # ---- load x, build xT ----------------------------------------------
    x_v = x_d.ap().rearrange("(n p) c -> p n c", p=P)  # [128, 32, 65]
    x_nat = const.tile([P, NT, CA], FP32, tag="xnat")
    for i in range(8):
        nc.sync.dma_start(x_nat[:, i * 4:(i + 1) * 4, :], x_v[:, i * 4:(i + 1) * 4, :])

    xT = const.tile([P, T], BF16, tag="xT")  # rows: 0:C x.T, C ones, rest 0
    # zero the pad rows (64:128); the ones row (64) is rewritten by the
    # transpose copies below.  gpsimd wants 32-aligned start partitions.
    nc.gpsimd.memset(xT[C:P, :], 0.0)
    for g in range(T // TB):
        psx = ps_big.tile([P, TB], FP32, tag="big")
        for j in range(TB // P):
            idx = g * (TB // P) + j
            nc.tensor.transpose(psx[0:CA, j * P:(j + 1) * P], x_nat[:, idx, :], ident)
        nc.vector.tensor_copy(xT[0:CA, g * TB:(g + 1) * TB], psx[0:CA, :])

    # ---- projections ----------------------------------------------------
    # qT[d, s] over all s; kT[d, t] over local t; v_aug[s, C+1] over all s.
    qt = []
    for i in range(T // SB):
        ps = ps_big.tile([P, SB], FP32, tag="big")
        nc.tensor.matmul(ps, lhsT=wq, rhs=xT[:, i * SB:(i + 1) * SB],
                         start=True, stop=True)
        q_sb = const.tile([P, SB], BF16, tag=f"qt{i}")
        nc.vector.tensor_copy(q_sb, ps)
        qt.append(q_sb)

    kt = []
    for i in range(T_LOC // TB):
        k_sb = const.tile([P, TB], BF16, tag=f"kt{i}")
        for j in range(TB // SB):
            ps = ps_big.tile([P, SB], FP32, tag="big")
            nc.tensor.matmul(ps, lhsT=wk,
                             rhs=xT[:, i * TB + j * SB:i * TB + (j + 1) * SB],
                             start=True, stop=True)
            nc.vector.tensor_copy(k_sb[:, j * SB:(j + 1) * SB], ps)
        kt.append(k_sb)

    va = []
    for g in range(NT // 8):
        ps = ps_big.tile([P, 8 * C], FP32, tag="big")
        for j in range(8):
            idx = g * 8 + j
            nc.tensor.matmul(ps[:, j * C:(j + 1) * C],
                             lhsT=xT[:, idx * P:(idx + 1) * P], rhs=wv[:, 0:C],
                             start=True, stop=True)
        v_sb = const.tile([P, 8, P], BF16, tag=f"va{g}")
        nc.vector.tensor_copy(v_sb[:, :, 0:C], ps.rearrange("p (n c) -> p n c", c=C))
        nc.vector.memset(v_sb[:, :, C:CA], 1.0)
        nc.vector.memset(v_sb[:, :, CA:P], 0.0)
        va.append(v_sb)

    # ---- flash attention main loop --------------------------------------
    # s-tile outer loop: per s-tile load qt/va stationary weights once and
    # stream both 1024-wide t-blocks; both attn accumulators are persistent
    # in PSUM.  Software-pipelined: scores for s-tile st+1 are emitted before
    # the attn matmuls of s-tile st so PE never waits on ACT's exp.
    out_v = out_d.ap().rearrange("(n p) c -> p n c", p=P)  # [128, 16, 64]

    po = [ps_o.tile([P, TB], FP32, tag=f"o{tb}", name="po") for tb in range(N_TB)]
    ex = [[None] * N_TB for _ in range(NT)]

    def scores(st):
        for tb in range(N_TB):
            pss = ps_big.tile([P, TB], FP32, tag="big", name="pss")
            for h in range(TB // SB):
                nc.tensor.matmul(
                    pss[:, h * SB:(h + 1) * SB],
                    lhsT=qt[st // 4][:, (st % 4) * P:(st % 4 + 1) * P],
                    rhs=kt[tb][:, h * SB:(h + 1) * SB], start=True, stop=True)
            e = expp.tile([P, TB], BF16, tag="ex", name="ex")
            nc.scalar.activation(e, pss, AF.Exp)
            ex[st][tb] = e

    def attn(st):
        for tb in range(N_TB):
            for h in range(TB // SB):  # matmul dst must stay in one PSUM bank
                nc.tensor.matmul(po[tb][:, h * SB:(h + 1) * SB],
                                 lhsT=va[st // 8][:, st % 8, :],
                                 rhs=ex[st][tb][:, h * SB:(h + 1) * SB],
                                 start=(st == 0), stop=(st == NT_MAIN - 1))

    scores(0)
    for st in range(1, NT_MAIN):
        scores(st)
        attn(st - 1)
    attn(NT_MAIN - 1)

    # ---- finalize: transpose [CA, 128] chunks back to [128, CA],
    # normalize, apply gamma, add residual, store.
    for tb in range(N_TB):
        osb = osbp.tile([P, TB], FP32, tag="osb")
        nc.vector.tensor_copy(osb, po[tb])
        for j in range(TB // P):
            # alternate psum slots: po[tb]'s slot is free once osb is copied
            if j % 2 == 0:
                pt = ps_big.tile([P, P], FP32, tag="big", name="pt")
            else:
                pt = ps_o.tile([P, P], FP32, tag=f"o{tb}", name="pt")
            nc.tensor.transpose(pt, osb[:, j * P:(j + 1) * P], ident)
            rec = smallp.tile([P, 1], FP32, tag="rec")
            nc.vector.reciprocal(rec, pt[:, C:CA])
            grec = smallp.tile([P, 1], FP32, tag="grec")
            nc.vector.tensor_mul(grec, rec, g128)
            ot = outp.tile([P, C], FP32, tag="ot")
            nc.vector.tensor_scalar_mul(ot, pt[:, 0:C], grec)
            idx = tb * (TB // P) + j
            nc.vector.tensor_add(ot, ot, x_nat[:, idx, 0:C])
            nc.sync.dma_start(out_v[:, idx, :], ot)


def build():
    nc = bacc.Bacc("TRN2", target_bir_lowering=False, debug=False,
                   num_devices=N_CORES)
    x_d = nc.dram_tensor("x", [T, CA], FP32, kind="ExternalInput")
    wq_d = nc.dram_tensor("wq", [C, C], FP32, kind="ExternalInput")
    wk_d = nc.dram_tensor("wk", [C, C], FP32, kind="ExternalInput")
    wv_d = nc.dram_tensor("wv", [C, C], FP32, kind="ExternalInput")
    bq_d = nc.dram_tensor("bq", [C], FP32, kind="ExternalInput")
    bk_d = nc.dram_tensor("bk", [C], FP32, kind="ExternalInput")
    bv_d = nc.dram_tensor("bv", [C], FP32, kind="ExternalInput")
    g_d = nc.dram_tensor("gamma", [1], FP32, kind="ExternalInput")
    out_d = nc.dram_tensor("out", [T_LOC, C], FP32, kind="ExternalOutput")

    with tile.TileContext(nc) as tc, ExitStack() as ctx:
        _emit(tc, ctx, x_d, wq_d, wk_d, wv_d, bq_d, bk_d, bv_d, g_d, out_d)
    nc.compile()
    return nc


def make_in_maps(inputs, Wq, bq, Wk, bk, Wv, bv, gamma):
    """Shard the full inputs into per-core input maps."""
    x = np.asarray(inputs, dtype=np.float32).reshape(B, T, C)
    ones = np.ones((T, 1), dtype=np.float32)
    in_maps = []
    for core in range(N_CORES):
        b, h = divmod(core, HALVES)
        xb = x[b]
        if h:
            xb = np.concatenate([xb[h * T_LOC:], xb[:h * T_LOC]], axis=0)
        x_aug = np.ascontiguousarray(np.concatenate([xb, ones], axis=1))
        in_maps.append({
            "x": x_aug,
            "wq": np.asarray(Wq, np.float32), "bq": np.asarray(bq, np.float32),
            "wk": np.asarray(Wk, np.float32), "bk": np.asarray(bk, np.float32),
            "wv": np.asarray(Wv, np.float32), "bv": np.asarray(bv, np.float32),
            "gamma": np.asarray(gamma, np.float32),
        })
    return in_maps


def assemble(results):
    """Gather per-core [T_LOC, C] outputs into the full [B, 1, T, C]."""
    out = np.empty((B, 1, T, C), dtype=np.float32)
    for core in range(N_CORES):
        b, h = divmod(core, HALVES)
        out[b, 0, h * T_LOC:(h + 1) * T_LOC, :] = results[core]["out"]
    return out


_NC_CACHE = []


def kernel(inputs, Wq, bq, Wk, bk, Wv, bv, gamma):
    if not _NC_CACHE:
        _NC_CACHE.append(build())
    nc = _NC_CACHE[0]
    in_maps = make_in_maps(inputs, Wq, bq, Wk, bk, Wv, bv, gamma)
    res = run_bass_kernel_spmd(nc, in_maps, list(range(N_CORES)))
    return assemble(res.results)
